# revision 37
# baseline (speedup 1.0000x reference)
"""Trainium2 Bass kernel for nn_Bert_BiLSTM (segment-mean pooling + BiLSTM).

Strategy (8 NeuronCores, data-parallel over batch, Bc=8 per core):
  Phase A (pooling): pooledT[d,w] = hidden[t,d]^T @ M_scaled[t,w] via fp32r
      matmuls, where M_scaled is the host-built one-hot(word_ids)/count
      matrix (index preprocessing only).
  Phase B (projection): pre[g,w] = w_ih^T @ pooledT (fp32r) + bias, stored
      bf16. Quarter (0,q0)+(1,q3) computed up front; the remaining items are
      dripped one gate-chunk at a time into the scan's PE idle windows.
  Phase C (scan): 256 sequential LSTM steps per direction, both directions
      interleaved anti-phase on each core (forced into anti-phase at t=0 by
      ~1us of projection matmuls between the first two bursts). Gates in
      [G-part, B-free] layout: g gates in their own psum bank so tanh(g)
      starts mid-burst; pre_t injected via identity matmuls (open the
      accumulation groups off the h-critical path); 16 w_hh matmuls
      accumulate on top. Elementwise on ACT/DVE/GpSimd, h write split by
      kt half so the next step's k0 matmuls start earlier.
  Phase D: PE-transpose h history to [w, h] layout and DMA out; everything
      whose slots are final is dripped through the last scan block, the
      rest goes at the end as sample-pairs with DMA issue rotated across
      engine queues.

Host side: shard batch, build M_scaled (bf16), permute gates to [i,f,o,g]
order, cast weights, assemble/concat outputs.
"""

import os
import sys

for _p in ("/opt/trn_rl_repo", "/root/.axon_site/_ro/trn_rl_repo"):
    if os.path.isdir(_p) and _p not in sys.path:
        sys.path.append(_p)

import numpy as np
import ml_dtypes

NCORES = 8
BC = 8          # batch per core
T = 512
D = 768
W = 256
H = 256
G = 1024        # 4*H
NT = T // 128   # 4 t-tiles
ND = D // 128   # 6 d-chunks
NG = G // 128   # 8 gate chunks (per direction)
KT = H // 128   # 2 h-chunks

_NC_CACHE = {}


def build_nc():
    """Build and compile the per-core Bass program (SPMD, same on all cores)."""
    import concourse.bacc as bacc
    import concourse.tile as tile
    from concourse import mybir
    from concourse.masks import make_identity

    f32 = mybir.dt.float32
    f32r = mybir.dt.float32r
    bf16 = mybir.dt.bfloat16
    AF = mybir.ActivationFunctionType
    ALU = mybir.AluOpType

    nc = bacc.Bacc("TRN2", target_bir_lowering=False, debug=False,
                   enable_asserts=False, num_devices=NCORES)

    hs = nc.dram_tensor("hs", [BC, NT, 128, D], bf16, kind="ExternalInput")
    msc = nc.dram_tensor("msc", [BC, NT, 128, W], bf16, kind="ExternalInput")
    wih = nc.dram_tensor("wih", [2, ND, 128, G], bf16, kind="ExternalInput")
    whh = nc.dram_tensor("whh", [2, KT, 128, G], bf16, kind="ExternalInput")
    bias = nc.dram_tensor("bias", [2 * NG, 128], f32, kind="ExternalInput")
    outf = nc.dram_tensor("outf", [BC, W, H], f32, kind="ExternalOutput")
    outb = nc.dram_tensor("outb", [BC, W, H], f32, kind="ExternalOutput")

    with tile.TileContext(nc) as tc:
        from contextlib import ExitStack
        ctx = ExitStack()
        with ctx:
            const = ctx.enter_context(tc.tile_pool(name="const", bufs=1))
            whh_sb = const.tile([128, 2, KT, G], bf16)
            nc.sync.dma_start(out=whh_sb, in_=whh.ap().rearrange("d k p g -> p d k g"))
            bias_sb = const.tile([128, 2 * NG], f32)
            nc.sync.dma_start(out=bias_sb, in_=bias.ap().rearrange("n p -> p n"))
            ident = const.tile([128, 128], bf16)
            make_identity(nc, ident)
            ident_pre = const.tile([128, 128], bf16)
            make_identity(nc, ident_pre)

            pooledT = const.tile([128, BC, ND, W], bf16)    # 24KB/part
            pre = const.tile([128, 2, W, NG, BC], bf16)     # 64KB/part
            hh = const.tile([128, 2, KT, BC, W + 1], bf16)  # h history
            cc = const.tile([128, 2, KT, BC], f32)

            # ---- Phase A: pooling ----
            with tc.tile_pool(name="hsst", bufs=3) as hsp, \
                 tc.tile_pool(name="mscst", bufs=2) as mscp, \
                 tc.tile_pool(name="psA", bufs=6, space="PSUM") as psA:
                for b in range(BC):
                    hst = []
                    msct = []
                    for tt in range(NT):
                        ht = hsp.tile([128, D], bf16, tag=f"hs{tt}")
                        nc.sync.dma_start(out=ht, in_=hs.ap()[b, tt])
                        hst.append(ht)
                        mt = mscp.tile([128, W], bf16, tag=f"ms{tt}")
                        nc.sync.dma_start(out=mt, in_=msc.ap()[b, tt])
                        msct.append(mt)
                    for dc in range(ND):
                        pps = psA.tile([128, W], f32)
                        for tt in range(NT):
                            nc.tensor.matmul(
                                out=pps,
                                lhsT=hst[tt][:, dc * 128:(dc + 1) * 128],
                                rhs=msct[tt],
                                start=(tt == 0), stop=(tt == NT - 1))
                        if (b * ND + dc) % 2 == 0:
                            nc.scalar.copy(pooledT[:, b, dc, :], pps)
                        else:
                            nc.vector.tensor_copy(pooledT[:, b, dc, :], pps)

            # scan pools first so the proj/psD pool stacks can close in
            # LIFO order around them
            bc_ctx = ctx.enter_context(ExitStack())
            psG = bc_ctx.enter_context(tc.tile_pool(name="psG", bufs=2, space="PSUM"))
            psS = bc_ctx.enter_context(tc.tile_pool(name="psS", bufs=4, space="PSUM"))
            sp = bc_ctx.enter_context(tc.tile_pool(name="sp", bufs=3))
            gp = bc_ctx.enter_context(tc.tile_pool(name="gp", bufs=3))
            tp = bc_ctx.enter_context(tc.tile_pool(name="tp", bufs=3))
            thp = bc_ctx.enter_context(tc.tile_pool(name="thp", bufs=3))

            # ---- Phase B: projection; (0,q0)+(1,q3) up front, rest dripped ----
            pb_ctx = ExitStack()
            wihp = pb_ctx.enter_context(tc.tile_pool(name="wihp", bufs=1))
            psB = pb_ctx.enter_context(tc.tile_pool(name="psB", bufs=2, space="PSUM"))
            wih_f = wihp.tile([128, ND, G], bf16, tag="wf")
            nc.sync.dma_start(out=wih_f, in_=wih.ap()[0].rearrange("c p g -> p c g"))
            wih_b = wihp.tile([128, ND, G], bf16, tag="wb")
            nc.sync.dma_start(out=wih_b, in_=wih.ap()[1].rearrange("c p g -> p c g"))

            def proj_item(di, wq, gc, sink):
                """One gate-chunk of one w-quarter: 6 MMs (N=512) + bias add."""
                for _ in proj_item_gen(di, wq, gc, sink):
                    pass

            def proj_item_gen(di, wq, gc, sink):
                """Generator form: yields after each MM so the drip can pump
                one matmul at a time into scan PE idle windows."""
                wih_sb = wih_f if di == 0 else wih_b
                ppj = psB.tile([128, BC, 64], f32)   # 1 bank (512 f32)
                for dc in range(ND):
                    nc.tensor.matmul(
                        out=ppj,
                        lhsT=wih_sb[:, dc, gc * 128:(gc + 1) * 128],
                        rhs=pooledT[:, :, dc, wq * 64:(wq + 1) * 64],
                        start=(dc == 0), stop=(dc == ND - 1))
                    yield
                bcol = bias_sb[:, di * NG + gc: di * NG + gc + 1]
                # pre is (w, gc, b)-ordered; psum is (b, w)
                dst = pre[:, di, wq * 64:(wq + 1) * 64, gc, :]
                src_ap = ppj.rearrange("p b w -> p w b")
                if sink == 0:
                    nc.vector.tensor_scalar(dst, src_ap, bcol, None, ALU.add)
                else:
                    nc.scalar.activation(dst, src_ap, AF.Identity,
                                         bias=bcol, scale=1.0)
                yield

            # ---- Phase C: the LSTM scan ----
            nc.vector.memset(hh[:, 0, :, :, 0], 0.0)     # fwd h_{-1} = 0
            nc.vector.memset(hh[:, 1, :, :, W], 0.0)     # bwd h_{W} = 0
            nc.vector.memset(cc, 0.0)

            def scan_mm(t, di):
                tf = t if di == 0 else W - 1 - t
                rslot = tf if di == 0 else tf + 1
                wslot = tf + 1 if di == 0 else tf
                # one psum BANK per step-dir but TWO accumulation groups in
                # it (disjoint slices): g gates stop early so tanh(g) starts
                # mid-burst. Single-tag pool -> bufs=4 fits the bank budget,
                # keeping all WAR reuse same-chain (2 steps back) instead of
                # cross-chain (1.5 steps), which decouples the chain phases.
                # Both pre-injection MMs first: no h dependency, so they run
                # during the previous step's elementwise tail.
                ps_g = psG.tile([128, KT, BC], f32, tag="psg")
                nc.tensor.matmul(out=ps_g, lhsT=ident_pre,
                                 rhs=pre[:, di, tf, 6:8, :],
                                 start=True, stop=False)
                ps_s = psS.tile([128, 6, BC], f32, tag="psifo")
                nc.tensor.matmul(out=ps_s, lhsT=ident_pre,
                                 rhs=pre[:, di, tf, 0:6, :],
                                 start=True, stop=False)
                # kt-outer: all k0 matmuls first so they're gated only on
                # the h0 half-write of the previous step
                for kt in range(KT):
                    for j, gc in enumerate((6, 7)):
                        nc.tensor.matmul(
                            out=ps_g[:, j, :],
                            lhsT=whh_sb[:, di, kt, gc * 128:(gc + 1) * 128],
                            rhs=hh[:, di, kt, :, rslot],
                            start=False, stop=(gc == 7 and kt == KT - 1))
                    for gc in range(6):
                        nc.tensor.matmul(
                            out=ps_s[:, gc, :],
                            lhsT=whh_sb[:, di, kt, gc * 128:(gc + 1) * 128],
                            rhs=hh[:, di, kt, :, rslot],
                            start=False, stop=(gc == 5 and kt == KT - 1))
                return (di, ps_g, ps_s, wslot)

            def scan_ew(st):
                di, ps_g, ps_s, wslot = st
                g = gp.tile([128, KT, BC], f32)
                nc.scalar.activation(g, ps_g, AF.Tanh)
                s = sp.tile([128, 6, BC], f32)
                nc.scalar.activation(s, ps_s, AF.Sigmoid)
                tmp = tp.tile([128, KT, BC], f32)
                nc.gpsimd.tensor_mul(tmp, s[:, 0:2, :], g)
                nc.vector.tensor_mul(cc[:, di], s[:, 2:4, :], cc[:, di])
                nc.vector.tensor_add(cc[:, di], cc[:, di], tmp)
                th = thp.tile([128, KT, BC], f32)
                nc.scalar.activation(th, cc[:, di], AF.Tanh)
                # split h write by kt half: next step's k0 matmuls only wait
                # for the first half
                nc.vector.tensor_mul(hh[:, di, 0, :, wslot], s[:, 4, :], th[:, 0, :])
                nc.vector.tensor_mul(hh[:, di, 1, :, wslot], s[:, 5, :], th[:, 1, :])

            def emit_out(di, b, wc):
                odram = outf if di == 0 else outb
                base = 1 if di == 0 else 0
                pst = psD.tile([128, KT, 128], bf16)
                for kt in range(KT):
                    nc.tensor.transpose(
                        pst[:, kt, :],
                        hh[:, di, kt, b, base + wc * 128: base + (wc + 1) * 128],
                        ident)
                stage = stg.tile([128, KT * 128], f32)
                if (b + wc) % 2 == 0:
                    nc.scalar.copy(stage, pst)
                else:
                    nc.vector.tensor_copy(stage, pst)
                nc.sync.dma_start(
                    out=odram.ap()[b, wc * 128:(wc + 1) * 128, :],
                    in_=stage)

            def emit_half(di, b, wc, hf):
                """Emit a 64-wide w-slice whose h slots are already final."""
                odram = outf if di == 0 else outb
                base = 1 if di == 0 else 0
                lo = base + wc * 128 + hf * 64
                wlo = wc * 128 + hf * 64
                pst = psD.tile([64, 2, KT, 128], bf16, tag="pair")
                for kt in range(KT):
                    nc.tensor.transpose(
                        pst[:, 0, kt, :], hh[:, di, kt, b, lo:lo + 64], ident)
                stage = stg.tile([64, KT * 128], f32, tag="shalf")
                if (b + hf) % 2 == 0:
                    nc.scalar.copy(stage, pst[:, 0])
                else:
                    nc.vector.tensor_copy(stage, pst[:, 0])
                nc.sync.dma_start(out=odram.ap()[b, wlo:wlo + 64, :], in_=stage)

            # Anti-phase emission: bwd's elementwise is emitted alongside
            # fwd's matmul burst and vice versa. Projection items are dripped
            # one gate-chunk per few steps into the scan's PE idle windows.
            for gc in range(NG):
                proj_item(0, 0, gc, gc % 2)
                proj_item(1, 3, gc, (gc + 1) % 2)
            # drip order: each quarter finishes well before its block starts
            drip = []
            for q, (qf, qb) in enumerate(((1, 2), (2, 1), (3, 0))):
                for gc in range(NG):
                    drip.append((0, qf, gc))
                    drip.append((1, qb, gc))
            emits = [(0, b, 0) for b in range(BC)] + [(1, b, 1) for b in range(BC)]

            # pump: one proj MM per insertion point, two points per step
            # (48 items x 7 ops = 336 ops over 2x192 = 384 slots)
            drip_iter = iter(drip)
            gen = None
            di_sink = 0

            def pump():
                nonlocal gen
                if gen is None:
                    nxt = next(drip_iter, None)
                    if nxt is None:
                        return
                    di, wq, gc = nxt
                    # bias-adds stay off the Scalar engine during the scan:
                    # a 600ns Identity there delays the sigmoid/tanh queue
                    gen = proj_item_gen(di, wq, gc, 0)
                if next(gen, "end") == "end":
                    gen = None

            pend_b = None
            for t in range(W):
                if t == 192:
                    pb_ctx.close()
                    psD = bc_ctx.enter_context(
                        tc.tile_pool(name="psD", bufs=1, space="PSUM"))
                    stg = bc_ctx.enter_context(tc.tile_pool(name="stg", bufs=4))
                st_f = scan_mm(t, 0)
                if t == 0:
                    # force the two chains into anti-phase from the start:
                    # ~1us of projection matmuls between the first fwd and
                    # bwd bursts offsets the chains by about half a period
                    for _ in range(5):
                        pump()
                if pend_b is not None:
                    scan_ew(pend_b)
                scan_ew(st_f)
                pend_b = scan_mm(t, 1)
                if t < 192:
                    pump()
                    pump()
                if t >= 192 and t % 4 == 0:
                    di, b, wc = emits[(t - 192) // 4]
                    emit_out(di, b, wc)
                if t >= 192 and t % 4 == 2:
                    k = (t - 194) // 4
                    if k < BC:
                        emit_half(0, k, 1, 0)     # fwd w 128..191
                    else:
                        emit_half(1, k - BC, 0, 1)  # bwd w 64..127
            scan_ew(pend_b)

            # ---- Phase D (part 2): remaining output slices (these cover the
            # final scan steps' slots, so they can only run at the end).
            # Two samples per stage/DMA, DMA issue rotated across engine
            # queues so the 16 transfers don't serialize on Sync. ----
            dma_engs = [nc.sync, nc.scalar, nc.gpsimd]
            for pi, bb in enumerate(range(0, BC, 2)):
                for ei, (di, wc, hf) in enumerate(((0, 1, 1), (1, 0, 0))):
                    odram = outf if di == 0 else outb
                    base = 1 if di == 0 else 0
                    lo = base + wc * 128 + hf * 64
                    wlo = wc * 128 + hf * 64
                    pst = psD.tile([64, 2, KT, 128], bf16, tag="pair")
                    for j in range(2):
                        for kt in range(KT):
                            nc.tensor.transpose(
                                pst[:, j, kt, :],
                                hh[:, di, kt, bb + j, lo:lo + 64], ident)
                    stage = stg.tile([64, 2, KT * 128], f32, tag="spair")
                    if (pi + ei) % 2 == 0:
                        nc.scalar.copy(stage, pst)
                    else:
                        nc.vector.tensor_copy(stage, pst)
                    eng = dma_engs[(pi * 2 + ei) % 3]
                    eng.dma_start(
                        out=odram.ap()[bb:bb + 2, wlo:wlo + 64, :]
                            .rearrange("b w h -> w b h"),
                        in_=stage)

    nc.compile()
    return nc


def get_nc():
    if "nc" not in _NC_CACHE:
        _NC_CACHE["nc"] = build_nc()
    return _NC_CACHE["nc"]


def prep_inputs(hidden_states, w_ih_f, w_hh_f, b_f, w_ih_b, w_hh_b, b_b,
                word_ids):
    """Host-side layout/dtype prep. Returns per-core input maps."""
    bf16 = ml_dtypes.bfloat16
    hidden_states = np.ascontiguousarray(hidden_states, dtype=np.float32)
    word_ids = np.asarray(word_ids)

    # scaled one-hot from the (index-only) word_ids
    M = (word_ids[:, :, None] == np.arange(W, dtype=word_ids.dtype)[None, None, :])
    M = M.astype(np.float32)
    counts = M.sum(axis=1)
    M *= (1.0 / np.maximum(counts, 1.0))[:, None, :]

    # gate permutation [i, f, g, o] -> [i, f, o, g]
    perm = np.concatenate([np.arange(0, 512), np.arange(768, 1024),
                           np.arange(512, 768)])

    def prep_dir(w_ih, w_hh, b):
        w_ih = np.asarray(w_ih, dtype=np.float32)[:, perm]
        w_hh = np.asarray(w_hh, dtype=np.float32)[:, perm]
        b = np.asarray(b, dtype=np.float32)[perm]
        return (w_ih.reshape(ND, 128, G).astype(bf16),
                w_hh.reshape(KT, 128, G).astype(bf16),
                b.reshape(NG, 128))

    wf, whf, bf_ = prep_dir(w_ih_f, w_hh_f, b_f)
    wb, whb, bb_ = prep_dir(w_ih_b, w_hh_b, b_b)
    wih_all = np.ascontiguousarray(np.stack([wf, wb]))
    whh_all = np.ascontiguousarray(np.stack([whf, whb]))
    bias_all = np.ascontiguousarray(np.concatenate([bf_, bb_], axis=0))

    in_maps = []
    for c in range(NCORES):
        sl = slice(c * BC, (c + 1) * BC)
        in_maps.append({
            "hs": np.ascontiguousarray(
                hidden_states[sl].reshape(BC, NT, 128, D)).astype(bf16),
            "msc": np.ascontiguousarray(M[sl].reshape(BC, NT, 128, W)).astype(bf16),
            "wih": wih_all,
            "whh": whh_all,
            "bias": bias_all,
        })
    return in_maps


def assemble_output(results):
    out = np.empty((NCORES * BC, W, 2 * H), dtype=np.float32)
    for c, r in enumerate(results):
        sl = slice(c * BC, (c + 1) * BC)
        out[sl, :, :H] = r["outf"]
        out[sl, :, H:] = r["outb"]
    return out


def kernel(hidden_states, w_ih_f, w_hh_f, b_f, w_ih_b, w_hh_b, b_b,
           word_ids, max_seq_len=None, **_unused):
    from concourse.bass_utils import run_bass_kernel_spmd

    in_maps = prep_inputs(hidden_states, w_ih_f, w_hh_f, b_f,
                          w_ih_b, w_hh_b, b_b, word_ids)
    nc = get_nc()
    res = run_bass_kernel_spmd(nc, in_maps, list(range(NCORES)))
    _NC_CACHE["last_exec_time_ns"] = res.exec_time_ns
    return assemble_output(res.results)
